# revision 14
# baseline (speedup 1.0000x reference)
"""BERT-base (12L, C=768, H=12, T=512, V=32000) forward on 8 Trainium2 NeuronCores.

Strategy: data-parallel over batch (B=8 -> 1 batch element per core).
Per core everything is computed with transposed activations xT [C, T]
(channel-major, 6 partition-tiles of [128, 512]), activations and weights in
bf16 (PSUM accumulation f32), which halves HBM traffic and SBUF pressure at
full PE speed (1 cyc/row):
  - Attention is pipelined over head PAIRS: the two heads of a pair live on
    partition halves [0:64] / [64:128]. Score matmuls (contraction D=64) are
    row-tiled (tile 64x128: even head rows 0-63, odd head rows 64-127) so the
    two heads' matmuls run concurrently on the PE array; the AV matmuls
    (M=64) are col-tiled (tile 128x64, outputs on partition halves of two
    PSUM banks). The next pair's Q/K/V projections are issued between a
    pair's score and AV matmuls so the PE stays busy during the softmax exps.
  - softmax is over the QUERY axis (reference softmax(dim=1) on [B,T,T]);
    scores are built transposed attT[k,q] = K @ Q^T (1/8 scale folded into
    the Exp activation) and normalization is folded into V rows (1/rowsum).
  - LayerNorm reduces over C = partition axis -> sums via matmul with a ones
    column; squares on DVE; rstd = exp(-0.5*ln(var+eps)) so the ACT engine
    only ever needs the natural_log_exp_and_others table set (no per-layer
    table swaps); mean/rstd broadcast over partitions with gpsimd.
  - decoder: logits[t, v] = x @ dec_W + dec_b, vocab streamed in 64 chunks of
    500 columns (dec_W bf16), logits written bf16 and upcast on host.
Embedding gather + positional add run on host (0.01% of FLOPs).
"""

import sys, os

sys.path.insert(0, "/opt/trn_rl_repo")

import numpy as np

L, H, C, D, FF, V, T, B = 12, 12, 768, 64, 3072, 32000, 512, 8
NC = C // 128        # 6 channel tiles (= head pairs)
NT = T // 128        # 4 token tiles
NFF = FF // 128      # 24 ffn tiles
VCW = 500            # vocab chunk width
VCN = V // VCW       # 64 vocab chunks
EPS = 1e-5
NCORES = 8

_ENGINE = {}


def _build_bass(n_layers=L, with_decoder=True, debug_xt=False):
    import concourse.bass as bass
    import concourse.mybir as mybir
    import concourse.tile as tile
    from concourse import bacc

    f32 = mybir.dt.float32
    f32r = mybir.dt.float32r
    bf16 = mybir.dt.bfloat16
    AF = mybir.ActivationFunctionType
    ALU = mybir.AluOpType

    nc = bacc.Bacc("TRN2", target_bir_lowering=False, debug=False,
                   num_devices=NCORES)

    # ---- DRAM I/O ----
    x0t_d = nc.dram_tensor("x0t", [C, T], bf16, kind="ExternalInput").ap()
    wq_d = nc.dram_tensor("wq", [L, C, C], bf16, kind="ExternalInput").ap()
    wk_d = nc.dram_tensor("wk", [L, C, C], bf16, kind="ExternalInput").ap()
    wv_d = nc.dram_tensor("wv", [L, C, C], bf16, kind="ExternalInput").ap()
    wo_d = nc.dram_tensor("wo", [L, C, C], bf16, kind="ExternalInput").ap()
    w1_d = nc.dram_tensor("w1", [L, NFF, 128, NC, 128], bf16, kind="ExternalInput").ap()
    w2_d = nc.dram_tensor("w2", [L, FF, C], bf16, kind="ExternalInput").ap()
    bo_d = nc.dram_tensor("bo", [L, C], f32, kind="ExternalInput").ap()
    b1_d = nc.dram_tensor("b1", [L, FF], f32, kind="ExternalInput").ap()
    b2_d = nc.dram_tensor("b2", [L, C], f32, kind="ExternalInput").ap()
    g1_d = nc.dram_tensor("g1", [L, C], f32, kind="ExternalInput").ap()
    be1_d = nc.dram_tensor("be1", [L, C], f32, kind="ExternalInput").ap()
    g2_d = nc.dram_tensor("g2", [L, C], f32, kind="ExternalInput").ap()
    be2_d = nc.dram_tensor("be2", [L, C], f32, kind="ExternalInput").ap()
    if with_decoder:
        decw_d = nc.dram_tensor("decw", [VCN, 128, NC, VCW], bf16, kind="ExternalInput").ap()
        decb_d = nc.dram_tensor("decb", [V], f32, kind="ExternalInput").ap()
        out_d = nc.dram_tensor("logits", [T, V], bf16, kind="ExternalOutput").ap()
    if debug_xt:
        xt_o_d = nc.dram_tensor("xt_out", [C, T], bf16, kind="ExternalOutput").ap()

    with tile.TileContext(nc) as tc:
        from contextlib import ExitStack

        with ExitStack() as octx:
            const = octx.enter_context(tc.tile_pool(name="const", bufs=1))
            xfp = octx.enter_context(tc.tile_pool(name="xfp", bufs=6))
            ctx = octx.enter_context(ExitStack())
            trunk = ctx.enter_context(tc.tile_pool(name="trunk", bufs=13))
            qkp = ctx.enter_context(tc.tile_pool(name="qkp", bufs=5))
            vvp = ctx.enter_context(tc.tile_pool(name="vvp", bufs=3))
            ocp = ctx.enter_context(tc.tile_pool(name="ocp", bufs=6))
            smp = ctx.enter_context(tc.tile_pool(name="smp", bufs=8))
            vsp = ctx.enter_context(tc.tile_pool(name="vsp", bufs=10))
            wpp = ctx.enter_context(tc.tile_pool(name="wpp", bufs=30))
            w1p = ctx.enter_context(tc.tile_pool(name="w1p", bufs=6))
            w2p = ctx.enter_context(tc.tile_pool(name="w2p", bufs=4))
            h1p = ctx.enter_context(tc.tile_pool(name="h1p", bufs=3))
            sqp = ctx.enter_context(tc.tile_pool(name="sqp", bufs=3))
            bcp = ctx.enter_context(tc.tile_pool(name="bcp", bufs=4))
            svp = ctx.enter_context(tc.tile_pool(name="svp", bufs=16))
            stp = ctx.enter_context(tc.tile_pool(name="stp", bufs=2))

            ones_f = const.tile([128, 1], f32, name="ones_f", tag="ones_f")
            nc.vector.memset(ones_f, 1.0)
            ones_r = const.tile([128, 1], f32r, name="ones_r", tag="ones_r")
            nc.scalar.copy(ones_r, ones_f)
            zerov = const.tile([128, 1], f32, name="zerov", tag="zerov")
            nc.vector.memset(zerov, 0.0)
            epsv = const.tile([1, 1], f32, name="epsv", tag="epsv")
            nc.vector.memset(epsv, EPS)

            # per-layer param vectors, chunk-major: [128, L, NC]
            def vec_tile(d_ap, n, tag):
                t = const.tile([128, L, n], f32, tag=tag)
                nc.sync.dma_start(
                    out=t, in_=d_ap.rearrange("l (m p) -> p l m", p=128))
                return t

            bo_v = vec_tile(bo_d, NC, "bo_v")
            b2_v = vec_tile(b2_d, NC, "b2_v")
            g1_v = vec_tile(g1_d, NC, "g1_v")
            be1_v = vec_tile(be1_d, NC, "be1_v")
            g2_v = vec_tile(g2_d, NC, "g2_v")
            be2_v = vec_tile(be2_d, NC, "be2_v")
            b1_v = vec_tile(b1_d, NFF, "b1_v")

            # layer-0 input
            xT = []
            x0r = x0t_d.rearrange("(m p) t -> p m t", p=128)
            for m in range(NC):
                t = trunk.tile([128, T], bf16, name="xT", tag="xT")
                nc.sync.dma_start(out=t, in_=x0r[:, m, :])
                xT.append(t)

            def layernorm(res, g_v, be_v, l, out_pool=None):
                """res: list of NC [128,T] f32r tiles -> new bf16 tiles.

                Processed in two T/2 halves: half B's PE sums overlap half
                A's stats/broadcast/apply chain, shrinking the per-LN serial
                bubble.
                """
                pool = out_pool if out_pool is not None else trunk
                TH = T // 2
                out = [pool.tile([128, T], bf16, name="xT", tag="xT")
                       for _ in range(NC)]
                with tc.tile_pool(name="ps_ln", bufs=4, space="PSUM") as psl:
                    for h2 in range(2):
                        lo = h2 * TH
                        ps_mu = psl.tile([1, TH], f32, name="ln", tag="ln")
                        ps_sq = psl.tile([1, TH], f32, name="ln", tag="ln")
                        for m in range(NC):
                            sq = sqp.tile([128, TH], f32r, name="sq", tag="sq")
                            nc.vector.tensor_mul(sq, res[m][:, lo:lo + TH],
                                                 res[m][:, lo:lo + TH])
                            nc.tensor.matmul(ps_mu, ones_r,
                                             res[m][:, lo:lo + TH],
                                             start=(m == 0), stop=(m == NC - 1))
                            nc.tensor.matmul(ps_sq, ones_r, sq,
                                             start=(m == 0), stop=(m == NC - 1))
                        nmu = stp.tile([1, TH], f32r, name="st", tag="st")
                        nc.scalar.mul(nmu, ps_mu, -1.0 / C)
                        nmu_b = bcp.tile([128, TH], f32r, name="bc", tag="bc")
                        nc.gpsimd.partition_broadcast(nmu_b, nmu)
                        # C*var = ps_sq - C*mu^2 ; rstd = exp(-.5*ln(var+eps))
                        aa = stp.tile([1, TH], f32, name="stf", tag="stf")
                        nc.vector.tensor_mul(aa, nmu, nmu)
                        bvar = stp.tile([1, TH], f32, name="stf", tag="stf")
                        nc.vector.scalar_tensor_tensor(
                            out=bvar, in0=aa, scalar=-float(C), in1=ps_sq,
                            op0=ALU.mult, op1=ALU.add)
                        lnv = stp.tile([1, TH], f32, name="stf", tag="stf")
                        nc.scalar.activation(lnv, bvar, AF.Ln,
                                             bias=epsv[:, :], scale=1.0 / C)
                        rstd = stp.tile([1, TH], f32r, name="st", tag="st")
                        nc.scalar.activation(rstd, lnv, AF.Exp,
                                             bias=zerov[:1, :], scale=-0.5)
                        rstd_b = bcp.tile([128, TH], f32r, name="bc", tag="bc")
                        nc.gpsimd.partition_broadcast(rstd_b, rstd)
                        for m in range(NC):
                            u = sqp.tile([128, TH], f32r, name="sq", tag="sq")
                            nc.vector.tensor_add(u, res[m][:, lo:lo + TH],
                                                 nmu_b)
                            nc.vector.tensor_mul(u, u, rstd_b)
                            nc.scalar.activation(out[m][:, lo:lo + TH], u,
                                                 AF.Identity,
                                                 bias=be_v[:, l, m:m + 1],
                                                 scale=g_v[:, l, m:m + 1])
                return out

            for l in range(n_layers):
                wq_r = wq_d[l].rearrange("(m p) n -> p m n", p=128)
                wk_r = wk_d[l].rearrange("(m p) n -> p m n", p=128)
                wv_r = wv_d[l].rearrange("(m p) n -> p m n", p=128)
                wo_r = wo_d[l].rearrange("(m p) n -> p m n", p=128)

                def load_w(r):
                    ts = []
                    for m in range(NC):
                        t = wpp.tile([128, C], bf16, name="wp", tag="wp")
                        nc.sync.dma_start(out=t, in_=r[:, m, :])
                        ts.append(t)
                    return ts

                wqt = load_w(wq_r)
                wkt = load_w(wk_r)
                wvt = load_w(wv_r)

                # ---- attention, pipelined over head pairs ----
                QT, KT, Vv = [None] * NC, [None] * NC, [None] * NC
                OC = []
                res1 = []
                with tc.tile_pool(name="ps_sc", bufs=3, space="PSUM") as pssc, \
                     tc.tile_pool(name="ps_o", bufs=2, space="PSUM") as pso, \
                     ExitStack() as qctx:
                    psqk = qctx.enter_context(
                        tc.tile_pool(name="ps_qk", bufs=2, space="PSUM"))
                    psv = qctx.enter_context(
                        tc.tile_pool(name="ps_v", bufs=1, space="PSUM"))

                    def do_qk(hi):
                        pq = psqk.tile([128, T], f32, name="a", tag="a")
                        for ct in range(NC):
                            nc.tensor.matmul(
                                pq, wqt[ct][:, hi * 128:(hi + 1) * 128],
                                xT[ct], start=(ct == 0), stop=(ct == NC - 1))
                        q = qkp.tile([128, T], bf16, name="qt", tag="qt")
                        nc.vector.tensor_copy(q, pq)
                        QT[hi] = q
                        pk = psqk.tile([128, T], f32, name="a", tag="a")
                        for ct in range(NC):
                            nc.tensor.matmul(
                                pk, wkt[ct][:, hi * 128:(hi + 1) * 128],
                                xT[ct], start=(ct == 0), stop=(ct == NC - 1))
                        k = qkp.tile([128, T], bf16, name="kt", tag="kt")
                        nc.vector.tensor_copy(k, pk)
                        KT[hi] = k

                    def do_v(hi):
                        pv = psv.tile([128, NT, 128], f32, name="v", tag="v")
                        for tn in range(NT):
                            for ct in range(NC):
                                nc.tensor.matmul(
                                    pv[:, tn, :],
                                    xT[ct][:, tn * 128:(tn + 1) * 128],
                                    wvt[ct][:, hi * 128:(hi + 1) * 128],
                                    start=(ct == 0), stop=(ct == NC - 1))
                        v = vvp.tile([128, NT, 128], f32r, name="vv", tag="vv")
                        nc.vector.tensor_copy(v, pv)
                        Vv[hi] = v

                    def emit_scores(hi):
                        sms, vss = [], []
                        for kt in range(NT):
                            pas = []
                            for half in range(2):
                                ho = half * 64
                                p = pssc.tile([128, T], f32, name="att",
                                              tag="att")
                                nc.tensor.matmul(
                                    p,
                                    KT[hi][ho:ho + 64, kt * 128:(kt + 1) * 128],
                                    QT[hi][ho:ho + 64, :],
                                    start=True, stop=True)
                                pas.append(p)
                            for half in range(2):
                                ho = half * 64
                                sm = smp.tile([128, T], f32r, name="sm",
                                              tag="sm")
                                ssum = svp.tile([128, 1], f32, name="sv",
                                                tag="sv")
                                nc.scalar.activation(sm, pas[half], AF.Exp,
                                                     bias=zerov[:, :],
                                                     scale=0.125,
                                                     accum_out=ssum)
                                isum = svp.tile([128, 1], f32, name="sv",
                                                tag="sv")
                                nc.vector.reciprocal(isum, ssum)
                                vs = vsp.tile([128, 64], f32r, name="vs",
                                              tag="vs")
                                nc.vector.tensor_scalar_mul(
                                    vs, Vv[hi][:, kt, ho:ho + 64], isum)
                                sms.append(sm)
                                vss.append(vs)
                        return sms, vss

                    def emit_av(hi, sms, vss):
                        oc = ocp.tile([128, T], bf16, name="oc", tag="oc")
                        for half in range(2):
                            ho = half * 64
                            po = pso.tile([64, T], f32, name="oh", tag="oh")
                            for kt in range(NT):
                                nc.tensor.matmul(po, vss[kt * 2 + half],
                                                 sms[kt * 2 + half],
                                                 start=(kt == 0),
                                                 stop=(kt == NT - 1))
                            nc.vector.tensor_copy(oc[ho:ho + 64, :], po)
                        OC.append(oc)

                    do_qk(0)
                    do_v(0)
                    for hi in range(NC - 1):
                        sms, vss = emit_scores(hi)
                        do_qk(hi + 1)
                        do_v(hi + 1)
                        if hi == 0:
                            wot = load_w(wo_r)  # prefetch Wo
                        emit_av(hi, sms, vss)
                    sms5, vss5 = emit_scores(NC - 1)
                    qctx.close()  # free ps_qk/ps_v banks for the projection

                    # out proj: partial accumulation (ct<5) fills the PE
                    # while the last pair's exps run on ACT
                    with tc.tile_pool(name="ps_c", bufs=3, space="PSUM") as psc:
                        NFILL = 3

                        def stt_res(py, m):
                            r = trunk.tile([128, T], f32r, name="res",
                                           tag="res", bufs=7)
                            nc.vector.scalar_tensor_tensor(
                                out=r, in0=py,
                                scalar=bo_v[:, l, m:m + 1], in1=xT[m],
                                op0=ALU.add, op1=ALU.add)
                            res1.append(r)

                        pys = []
                        for m in range(NFILL):
                            py = psc.tile([128, T], f32, name="c", tag="c")
                            for ct in range(NC - 1):
                                nc.tensor.matmul(
                                    py, wot[ct][:, m * 128:(m + 1) * 128],
                                    OC[ct], start=(ct == 0), stop=False)
                            pys.append(py)
                        emit_av(NC - 1, sms5, vss5)
                        for m in range(NFILL):
                            nc.tensor.matmul(
                                pys[m], wot[NC - 1][:, m * 128:(m + 1) * 128],
                                OC[NC - 1], start=False, stop=True)
                            stt_res(pys[m], m)
                        for m in range(NFILL, NC):
                            py = psc.tile([128, T], f32, name="c", tag="c")
                            for ct in range(NC):
                                nc.tensor.matmul(
                                    py, wot[ct][:, m * 128:(m + 1) * 128],
                                    OC[ct], start=(ct == 0),
                                    stop=(ct == NC - 1))
                            stt_res(py, m)
                xln = layernorm(res1, g1_v, be1_v, l)

                # ---- FFN ----
                w2_r = w2_d[l].rearrange("(hh p) n -> p hh n", p=128)
                res2 = []
                with tc.tile_pool(name="ps_acc", bufs=6, space="PSUM") as psd, \
                     tc.tile_pool(name="ps_h1", bufs=2, space="PSUM") as psh:
                    acc = [psd.tile([128, T], f32, name="acc", tag="acc")
                           for _ in range(NC)]
                    for hh in range(NFF):
                        w1t = w1p.tile([128, NC, 128], bf16, name="w1", tag="w1")
                        nc.sync.dma_start(out=w1t, in_=w1_d[l, hh])
                        w2t = w2p.tile([128, C], bf16, name="w2", tag="w2")
                        nc.sync.dma_start(out=w2t, in_=w2_r[:, hh, :])
                        ph = psh.tile([128, T], f32, name="h1", tag="h1")
                        for ct in range(NC):
                            nc.tensor.matmul(ph, w1t[:, ct, :], xln[ct],
                                             start=(ct == 0), stop=(ct == NC - 1))
                        h1 = h1p.tile([128, T], bf16, name="h1s", tag="h1s")
                        nc.scalar.activation(h1, ph, AF.Relu,
                                             bias=b1_v[:, l, hh:hh + 1],
                                             scale=1.0)
                        for m in range(NC):
                            nc.tensor.matmul(acc[m], w2t[:, m * 128:(m + 1) * 128],
                                             h1, start=(hh == 0),
                                             stop=(hh == NFF - 1))
                    for m in range(NC):
                        r = trunk.tile([128, T], f32r, name="res", tag="res",
                                       bufs=7)
                        nc.vector.scalar_tensor_tensor(
                            out=r, in0=acc[m],
                            scalar=b2_v[:, l, m:m + 1], in1=xln[m],
                            op0=ALU.add, op1=ALU.add)
                        res2.append(r)
                last = (l == n_layers - 1)
                xT = layernorm(res2, g2_v, be2_v, l,
                               out_pool=(xfp if last else None))

            xf = xT
            ctx.close()

            if debug_xt:
                xo_r = xt_o_d.rearrange("(m p) t -> p m t", p=128)
                for m in range(NC):
                    nc.sync.dma_start(out=xo_r[:, m, :], in_=xf[m])

            # ---- decoder ----
            if with_decoder:
                with tc.tile_pool(name="dwp", bufs=4) as dwp, \
                     tc.tile_pool(name="dbp", bufs=6) as dbp, \
                     tc.tile_pool(name="dop", bufs=8) as dop, \
                     tc.tile_pool(name="ps_d", bufs=6, space="PSUM") as psd2:
                    for vc in range(VCN):
                        dwt = dwp.tile([128, NC, VCW], bf16, name="dw", tag="dw")
                        nc.sync.dma_start(out=dwt, in_=decw_d[vc])
                        db1 = dbp.tile([1, VCW], f32, name="db1", tag="db1")
                        nc.sync.dma_start(
                            out=db1,
                            in_=decb_d[vc * VCW:(vc + 1) * VCW]
                            .rearrange("(a v) -> a v", a=1))
                        dbb = dbp.tile([128, VCW], f32, name="dbb", tag="dbb")
                        nc.gpsimd.partition_broadcast(dbb, db1)
                        for tn in range(NT):
                            pd = psd2.tile([128, VCW], f32, name="d", tag="d")
                            for m in range(NC):
                                nc.tensor.matmul(
                                    pd, xf[m][:, tn * 128:(tn + 1) * 128],
                                    dwt[:, m, :], start=(m == 0),
                                    stop=(m == NC - 1))
                            ot = dop.tile([128, VCW], bf16, name="do", tag="do")
                            nc.vector.tensor_add(ot, pd, dbb)
                            nc.sync.dma_start(
                                out=out_d[tn * 128:(tn + 1) * 128,
                                          vc * VCW:(vc + 1) * VCW],
                                in_=ot)

    nc.compile()
    return nc


def _get_engine(n_layers=L, with_decoder=True, debug_xt=False):
    key = (n_layers, with_decoder, debug_xt)
    if key in _ENGINE:
        return _ENGINE[key]

    import jax
    import jax.numpy as jnp
    from jax.sharding import Mesh, PartitionSpec, NamedSharding
    from jax.experimental.shard_map import shard_map
    import concourse.mybir as mybir
    from concourse import bass2jax
    from concourse.bass2jax import _bass_exec_p, install_neuronx_cc_hook

    # Persistent NEFF cache keyed on BIR bytes.
    if not getattr(bass2jax, "_neff_cache_installed", False):
        import hashlib, shutil
        _orig_compile = bass2jax.compile_bir_kernel

        def _cached_compile(ant_bir_str, compile_dir_path, neff_name="file.neff"):
            cache_dir = os.path.expanduser("~/.cache/bass_neff")
            os.makedirs(cache_dir, exist_ok=True)
            key = hashlib.sha256(
                ant_bir_str if isinstance(ant_bir_str, bytes)
                else ant_bir_str.encode()).hexdigest()
            hit = os.path.join(cache_dir, f"{key}.neff")
            out = os.path.join(compile_dir_path, neff_name)
            if os.path.exists(hit):
                shutil.copyfile(hit, out)
                return out
            res = _orig_compile(ant_bir_str, compile_dir_path, neff_name)
            try:
                shutil.copyfile(res, hit)
            except OSError:
                pass
            return res

        bass2jax.compile_bir_kernel = _cached_compile
        bass2jax._neff_cache_installed = True

    install_neuronx_cc_hook()
    nc = _build_bass(n_layers, with_decoder, debug_xt)

    partition_name = (nc.partition_id_tensor.name
                      if nc.partition_id_tensor else None)
    in_names, out_names, out_avals = [], [], []
    zero_shapes = []
    for alloc in nc.m.functions[0].allocations:
        if not isinstance(alloc, mybir.MemoryLocationSet):
            continue
        name = alloc.memorylocations[0].name
        if alloc.kind == "ExternalInput":
            if name != partition_name:
                in_names.append(name)
        elif alloc.kind == "ExternalOutput":
            out_names.append(name)
            shape = tuple(alloc.tensor_shape)
            dtype = mybir.dt.np(alloc.dtype)
            out_avals.append(jax.core.ShapedArray(shape, dtype))
            zero_shapes.append((shape, dtype))
    n_params = len(in_names)
    all_in_names = in_names + out_names
    if partition_name is not None:
        all_in_names = all_in_names + [partition_name]

    def _body(*args):
        operands = list(args)
        if partition_name is not None:
            operands.append(bass2jax.partition_id_tensor())
        outs = _bass_exec_p.bind(
            *operands,
            out_avals=tuple(out_avals),
            in_names=tuple(all_in_names),
            out_names=tuple(out_names),
            lowering_input_output_aliases=(),
            sim_require_finite=True,
            sim_require_nnan=True,
            nc=nc,
        )
        return tuple(outs)

    devices = jax.devices()[:NCORES]
    mesh = Mesh(np.asarray(devices), ("core",))
    sharded_inputs = {"x0t"}
    in_specs = tuple(
        PartitionSpec("core") if n in sharded_inputs else PartitionSpec()
        for n in in_names) + (PartitionSpec("core"),) * len(out_names)
    out_specs = (PartitionSpec("core"),) * len(out_names)
    sharded = jax.jit(shard_map(_body, mesh=mesh, in_specs=in_specs,
                                out_specs=out_specs, check_rep=False),
                      keep_unused=True)

    shard = NamedSharding(mesh, PartitionSpec("core"))
    repl = NamedSharding(mesh, PartitionSpec())
    in_shardings = {n: (shard if n in sharded_inputs else repl)
                    for n in in_names}

    def make_zeros():
        return [
            jax.device_put(
                np.zeros((NCORES * s[0], *s[1:]), dt), shard)
            for (s, dt) in zero_shapes
        ]

    eng = dict(nc=nc, in_names=in_names, out_names=out_names,
               out_avals=out_avals, sharded=sharded, mesh=mesh, shard=shard,
               in_shardings=in_shardings,
               make_zeros=make_zeros, zeros=None, dev_args=None,
               dev_args_key=None)
    _ENGINE[key] = eng
    return eng


def _host_prep(inputs):
    """Returns dict name -> per-core-stacked array [NCORES*d0, ...]."""
    import ml_dtypes
    bf = ml_dtypes.bfloat16

    ids = np.asarray(inputs["input_ids"])
    emb = np.asarray(inputs["emb"], dtype=np.float32)
    pos = np.asarray(inputs["pos"], dtype=np.float32)
    x0 = emb[ids] + pos[None, :T]                      # [B, T, C]
    x0t = np.ascontiguousarray(x0.transpose(0, 2, 1)).astype(bf)  # [B, C, T]

    Wq = np.asarray(inputs["Wq"], dtype=np.float32)
    Wk = np.asarray(inputs["Wk"], dtype=np.float32)
    Wv = np.asarray(inputs["Wv"], dtype=np.float32)
    wq = np.ascontiguousarray(Wq.transpose(0, 2, 1, 3).reshape(L, C, C))
    wk = np.ascontiguousarray(Wk.transpose(0, 2, 1, 3).reshape(L, C, C))
    wv = np.ascontiguousarray(Wv.transpose(0, 2, 1, 3).reshape(L, C, C))

    def bfc(x):
        return np.ascontiguousarray(np.asarray(x, dtype=np.float32)).astype(bf)

    def f32c(x):
        return np.ascontiguousarray(np.asarray(x, dtype=np.float32))

    shared = {
        "wq": wq.astype(bf), "wk": wk.astype(bf), "wv": wv.astype(bf),
        "wo": bfc(inputs["Wo"]),
        "w1": np.asarray(inputs["W1"], dtype=np.float32)
        .reshape(L, NC, 128, NFF, 128).transpose(0, 3, 2, 1, 4).astype(bf),
        "w2": bfc(inputs["W2"]), "bo": f32c(inputs["bo"]),
        "b1": f32c(inputs["b1"]), "b2": f32c(inputs["b2"]),
        "g1": f32c(inputs["ln1_g"]), "be1": f32c(inputs["ln1_b"]),
        "g2": f32c(inputs["ln2_g"]), "be2": f32c(inputs["ln2_b"]),
        "decw": np.asarray(inputs["dec_W"], dtype=np.float32)
        .reshape(NC, 128, VCN, VCW).transpose(2, 1, 0, 3).astype(bf),
        "decb": f32c(inputs["dec_b"]),
    }
    stacked = {"x0t": x0t.reshape(B * C, T)}
    stacked.update(shared)
    return stacked


def _run(eng, stacked, want=None):
    import jax
    key = tuple(id(stacked[name]) for name in eng["in_names"])
    if eng["dev_args_key"] != key:
        eng["dev_args"] = [
            jax.device_put(stacked[name], eng["in_shardings"][name])
            for name in eng["in_names"]]
        eng["dev_args_key"] = key
    if eng["zeros"] is None:
        eng["zeros"] = eng["make_zeros"]()
    out = eng["sharded"](*eng["dev_args"], *eng["zeros"])
    res = {}
    for i, name in enumerate(eng["out_names"]):
        if want is not None and name not in want:
            continue
        a = np.asarray(out[i])
        res[name] = a.reshape(NCORES, -1, *a.shape[1:])
    return res


_PREP_CACHE = {}


def kernel(**inputs):
    eng = _get_engine()
    pkey = tuple(id(inputs[k]) for k in sorted(inputs))
    stacked = _PREP_CACHE.get(pkey)
    if stacked is None:
        stacked = _host_prep(inputs)
        _PREP_CACHE.clear()
        _PREP_CACHE[pkey] = stacked
    res = _run(eng, stacked, want=("logits",))
    logits = res["logits"].reshape(NCORES, T, V)
    return logits.astype(np.float32)


if __name__ == "__main__":
    rng = np.random.default_rng(0)
    dummy = {
        "input_ids": rng.integers(0, V, (B, T)),
        "emb": rng.standard_normal((V, C), dtype=np.float32) * 0.02,
        "pos": rng.standard_normal((T, C), dtype=np.float32) * 0.02,
        "Wq": rng.standard_normal((L, H, C, D), dtype=np.float32) * 0.02,
        "Wk": rng.standard_normal((L, H, C, D), dtype=np.float32) * 0.02,
        "Wv": rng.standard_normal((L, H, C, D), dtype=np.float32) * 0.02,
        "Wo": rng.standard_normal((L, C, C), dtype=np.float32) * 0.02,
        "bo": np.zeros((L, C), np.float32),
        "ln1_g": np.ones((L, C), np.float32),
        "ln1_b": np.zeros((L, C), np.float32),
        "W1": rng.standard_normal((L, C, FF), dtype=np.float32) * 0.02,
        "b1": np.zeros((L, 4 * C), np.float32),
        "W2": rng.standard_normal((L, FF, C), dtype=np.float32) * 0.02,
        "b2": np.zeros((L, C), np.float32),
        "ln2_g": np.ones((L, C), np.float32),
        "ln2_b": np.zeros((L, C), np.float32),
        "dec_W": rng.standard_normal((C, V), dtype=np.float32) * 0.02,
        "dec_b": np.zeros((V,), np.float32),
    }
    out = kernel(**dummy)
    print("out", out.shape, out.dtype, float(np.abs(out).max()))


# revision 17
# speedup vs baseline: 1.0545x; 1.0545x over previous
"""BERT-base (12L, C=768, H=12, T=512, V=32000) forward on 8 Trainium2 NeuronCores.

Strategy: data-parallel over batch (B=8 -> 1 batch element per core).
Per core everything is computed with transposed activations xT [C, T]
(channel-major, 6 partition-tiles of [128, 512]), activations and weights in
bf16 (PSUM accumulation f32), which halves HBM traffic and SBUF pressure at
full PE speed (1 cyc/row):
  - Attention is pipelined over head PAIRS: the two heads of a pair live on
    partition halves [0:64] / [64:128]. Score matmuls (contraction D=64) are
    row-tiled (tile 64x128: even head rows 0-63, odd head rows 64-127) so the
    two heads' matmuls run concurrently on the PE array; the AV matmuls
    (M=64) are col-tiled (tile 128x64, outputs on partition halves of two
    PSUM banks). The next pair's Q/K/V projections are issued between a
    pair's score and AV matmuls so the PE stays busy during the softmax exps.
  - softmax is over the QUERY axis (reference softmax(dim=1) on [B,T,T]);
    scores are built transposed attT[k,q] = K @ Q^T (1/8 scale folded into
    the Exp activation) and normalization is folded into V rows (1/rowsum).
  - LayerNorm reduces over C = partition axis -> sums via matmul with a ones
    column; squares on DVE; rstd = exp(-0.5*ln(var+eps)) so the ACT engine
    only ever needs the natural_log_exp_and_others table set (no per-layer
    table swaps); mean/rstd broadcast over partitions with gpsimd.
  - decoder: logits[t, v] = x @ dec_W + dec_b, vocab streamed in 64 chunks of
    500 columns (dec_W bf16), logits written bf16 and upcast on host.
Embedding gather + positional add run on host (0.01% of FLOPs).
"""

import sys, os

sys.path.insert(0, "/opt/trn_rl_repo")

import numpy as np

L, H, C, D, FF, V, T, B = 12, 12, 768, 64, 3072, 32000, 512, 8
NC = C // 128        # 6 channel tiles (= head pairs)
NT = T // 128        # 4 token tiles
NFF = FF // 128      # 24 ffn tiles
VCW = 500            # vocab chunk width
VCN = V // VCW       # 64 vocab chunks
EPS = 1e-5
NCORES = 8

_ENGINE = {}


def _build_bass(n_layers=L, with_decoder=True, debug_xt=False):
    import concourse.bass as bass
    import concourse.mybir as mybir
    import concourse.tile as tile
    from concourse import bacc

    f32 = mybir.dt.float32
    f32r = mybir.dt.float32r
    bf16 = mybir.dt.bfloat16
    AF = mybir.ActivationFunctionType
    ALU = mybir.AluOpType

    nc = bacc.Bacc("TRN2", target_bir_lowering=False, debug=False,
                   num_devices=NCORES)

    # ---- DRAM I/O ----
    x0t_d = nc.dram_tensor("x0t", [C, T], bf16, kind="ExternalInput").ap()
    wq_d = nc.dram_tensor("wq", [L, C, C], bf16, kind="ExternalInput").ap()
    wk_d = nc.dram_tensor("wk", [L, C, C], bf16, kind="ExternalInput").ap()
    wv_d = nc.dram_tensor("wv", [L, C, C], bf16, kind="ExternalInput").ap()
    wo_d = nc.dram_tensor("wo", [L, C, C], bf16, kind="ExternalInput").ap()
    w1_d = nc.dram_tensor("w1", [L, NFF, 128, NC, 128], bf16, kind="ExternalInput").ap()
    w2_d = nc.dram_tensor("w2", [L, FF, C], bf16, kind="ExternalInput").ap()
    bo_d = nc.dram_tensor("bo", [L, C], f32, kind="ExternalInput").ap()
    b1_d = nc.dram_tensor("b1", [L, FF], f32, kind="ExternalInput").ap()
    b2_d = nc.dram_tensor("b2", [L, C], f32, kind="ExternalInput").ap()
    g1_d = nc.dram_tensor("g1", [L, C], f32, kind="ExternalInput").ap()
    be1_d = nc.dram_tensor("be1", [L, C], f32, kind="ExternalInput").ap()
    g2_d = nc.dram_tensor("g2", [L, C], f32, kind="ExternalInput").ap()
    be2_d = nc.dram_tensor("be2", [L, C], f32, kind="ExternalInput").ap()
    if with_decoder:
        decw_d = nc.dram_tensor("decw", [VCN, 128, NC, VCW], bf16, kind="ExternalInput").ap()
        decb_d = nc.dram_tensor("decb", [V], f32, kind="ExternalInput").ap()
        out_d = nc.dram_tensor("logits", [T, V], bf16, kind="ExternalOutput").ap()
    if debug_xt:
        xt_o_d = nc.dram_tensor("xt_out", [C, T], bf16, kind="ExternalOutput").ap()

    with tile.TileContext(nc) as tc:
        from contextlib import ExitStack

        with ExitStack() as octx:
            const = octx.enter_context(tc.tile_pool(name="const", bufs=1))
            xfp = octx.enter_context(tc.tile_pool(name="xfp", bufs=6))
            ctx = octx.enter_context(ExitStack())
            trunk = ctx.enter_context(tc.tile_pool(name="trunk", bufs=13))
            qkp = ctx.enter_context(tc.tile_pool(name="qkp", bufs=5))
            vvp = ctx.enter_context(tc.tile_pool(name="vvp", bufs=3))
            ocp = ctx.enter_context(tc.tile_pool(name="ocp", bufs=6))
            smp = ctx.enter_context(tc.tile_pool(name="smp", bufs=8))
            vsp = ctx.enter_context(tc.tile_pool(name="vsp", bufs=10))
            wpp = ctx.enter_context(tc.tile_pool(name="wpp", bufs=30))
            w1p = ctx.enter_context(tc.tile_pool(name="w1p", bufs=6))
            w2p = ctx.enter_context(tc.tile_pool(name="w2p", bufs=4))
            h1p = ctx.enter_context(tc.tile_pool(name="h1p", bufs=3))
            sqp = ctx.enter_context(tc.tile_pool(name="sqp", bufs=3))
            bcp = ctx.enter_context(tc.tile_pool(name="bcp", bufs=2))
            svp = ctx.enter_context(tc.tile_pool(name="svp", bufs=16))
            stp = ctx.enter_context(tc.tile_pool(name="stp", bufs=2))

            ones_f = const.tile([128, 1], f32, name="ones_f", tag="ones_f")
            nc.vector.memset(ones_f, 1.0)
            ones_r = const.tile([128, 1], f32r, name="ones_r", tag="ones_r")
            nc.scalar.copy(ones_r, ones_f)
            zerov = const.tile([128, 1], f32, name="zerov", tag="zerov")
            nc.vector.memset(zerov, 0.0)
            epsv = const.tile([1, 1], f32, name="epsv", tag="epsv")
            nc.vector.memset(epsv, EPS)

            # per-layer param vectors, chunk-major: [128, L, NC]
            def vec_tile(d_ap, n, tag):
                t = const.tile([128, L, n], f32, tag=tag)
                nc.sync.dma_start(
                    out=t, in_=d_ap.rearrange("l (m p) -> p l m", p=128))
                return t

            bo_v = vec_tile(bo_d, NC, "bo_v")
            b2_v = vec_tile(b2_d, NC, "b2_v")
            g1_v = vec_tile(g1_d, NC, "g1_v")
            be1_v = vec_tile(be1_d, NC, "be1_v")
            g2_v = vec_tile(g2_d, NC, "g2_v")
            be2_v = vec_tile(be2_d, NC, "be2_v")
            b1_v = vec_tile(b1_d, NFF, "b1_v")

            # layer-0 input
            xT = []
            x0r = x0t_d.rearrange("(m p) t -> p m t", p=128)
            for m in range(NC):
                t = trunk.tile([128, T], bf16, name="xT", tag="xT")
                nc.sync.dma_start(out=t, in_=x0r[:, m, :])
                xT.append(t)

            def layernorm(res, g_v, be_v, l, out_pool=None):
                """res: list of NC [128,T] bf16 tiles -> new bf16 tiles."""
                pool = out_pool if out_pool is not None else trunk
                with tc.tile_pool(name="ps_ln", bufs=2, space="PSUM") as psl:
                    ps_mu = psl.tile([1, T], f32, name="ln", tag="ln")
                    ps_sq = psl.tile([1, T], f32, name="ln", tag="ln")
                    for m in range(NC):
                        sq = sqp.tile([128, T], f32r, name="sq", tag="sq")
                        nc.vector.tensor_mul(sq, res[m], res[m])
                        nc.tensor.matmul(ps_mu, ones_r, res[m],
                                         start=(m == 0), stop=(m == NC - 1))
                        nc.tensor.matmul(ps_sq, ones_r, sq,
                                         start=(m == 0), stop=(m == NC - 1))
                    nmu = stp.tile([1, T], f32r, name="st", tag="st")
                    nc.scalar.mul(nmu, ps_mu, -1.0 / C)
                    nmu_b = bcp.tile([128, T], f32r, name="bc", tag="bc")
                    nc.gpsimd.partition_broadcast(nmu_b, nmu)
                    aa = stp.tile([1, T], f32, name="stf", tag="stf")
                    nc.vector.tensor_mul(aa, nmu, nmu)
                    bvar = stp.tile([1, T], f32, name="stf", tag="stf")
                    nc.vector.scalar_tensor_tensor(
                        out=bvar, in0=aa, scalar=-float(C), in1=ps_sq,
                        op0=ALU.mult, op1=ALU.add)
                    lnv = stp.tile([1, T], f32, name="stf", tag="stf")
                    nc.scalar.activation(lnv, bvar, AF.Ln, bias=epsv[:, :],
                                         scale=1.0 / C)
                    rstd = stp.tile([1, T], f32r, name="st", tag="st")
                    nc.scalar.activation(rstd, lnv, AF.Exp, bias=zerov[:1, :],
                                         scale=-0.5)
                    rstd_b = bcp.tile([128, T], f32r, name="bc", tag="bc")
                    nc.gpsimd.partition_broadcast(rstd_b, rstd)
                out = []
                for m in range(NC):
                    u = sqp.tile([128, T], f32r, name="sq", tag="sq")
                    nc.vector.tensor_add(u, res[m], nmu_b)
                    nc.vector.tensor_mul(u, u, rstd_b)
                    t1 = pool.tile([128, T], bf16, name="xT", tag="xT")
                    nc.scalar.activation(t1, u, AF.Identity,
                                         bias=be_v[:, l, m:m + 1],
                                         scale=g_v[:, l, m:m + 1])
                    out.append(t1)
                return out

            for l in range(n_layers):
                wq_r = wq_d[l].rearrange("(m p) n -> p m n", p=128)
                wk_r = wk_d[l].rearrange("(m p) n -> p m n", p=128)
                wv_r = wv_d[l].rearrange("(m p) n -> p m n", p=128)
                wo_r = wo_d[l].rearrange("(m p) n -> p m n", p=128)

                def load_w(r):
                    ts = []
                    for m in range(NC):
                        t = wpp.tile([128, C], bf16, name="wp", tag="wp")
                        nc.sync.dma_start(out=t, in_=r[:, m, :])
                        ts.append(t)
                    return ts

                wqt = load_w(wq_r)
                wkt = load_w(wk_r)
                wvt = load_w(wv_r)

                # ---- attention, pipelined over head pairs ----
                QT, KT, Vv = [None] * NC, [None] * NC, [None] * NC
                OC = []
                res1 = []
                with tc.tile_pool(name="ps_sc", bufs=3, space="PSUM") as pssc, \
                     tc.tile_pool(name="ps_o", bufs=2, space="PSUM") as pso, \
                     ExitStack() as qctx:
                    psqk = qctx.enter_context(
                        tc.tile_pool(name="ps_qk", bufs=2, space="PSUM"))
                    psv = qctx.enter_context(
                        tc.tile_pool(name="ps_v", bufs=1, space="PSUM"))

                    def do_qk(hi):
                        pq = psqk.tile([128, T], f32, name="a", tag="a")
                        for ct in range(NC):
                            nc.tensor.matmul(
                                pq, wqt[ct][:, hi * 128:(hi + 1) * 128],
                                xT[ct], start=(ct == 0), stop=(ct == NC - 1))
                        q = qkp.tile([128, T], bf16, name="qt", tag="qt")
                        nc.vector.tensor_copy(q, pq)
                        QT[hi] = q
                        pk = psqk.tile([128, T], f32, name="a", tag="a")
                        for ct in range(NC):
                            nc.tensor.matmul(
                                pk, wkt[ct][:, hi * 128:(hi + 1) * 128],
                                xT[ct], start=(ct == 0), stop=(ct == NC - 1))
                        k = qkp.tile([128, T], bf16, name="kt", tag="kt")
                        nc.vector.tensor_copy(k, pk)
                        KT[hi] = k

                    def do_v(hi):
                        pv = psv.tile([128, NT, 128], f32, name="v", tag="v")
                        for tn in range(NT):
                            for ct in range(NC):
                                nc.tensor.matmul(
                                    pv[:, tn, :],
                                    xT[ct][:, tn * 128:(tn + 1) * 128],
                                    wvt[ct][:, hi * 128:(hi + 1) * 128],
                                    start=(ct == 0), stop=(ct == NC - 1))
                        v = vvp.tile([128, NT, 128], f32r, name="vv", tag="vv")
                        nc.vector.tensor_copy(v, pv)
                        Vv[hi] = v

                    def emit_scores(hi):
                        sms, vss = [], []
                        for kt in range(NT):
                            pas = []
                            for half in range(2):
                                ho = half * 64
                                p = pssc.tile([128, T], f32, name="att",
                                              tag="att")
                                nc.tensor.matmul(
                                    p,
                                    KT[hi][ho:ho + 64, kt * 128:(kt + 1) * 128],
                                    QT[hi][ho:ho + 64, :],
                                    start=True, stop=True)
                                pas.append(p)
                            for half in range(2):
                                ho = half * 64
                                sm = smp.tile([128, T], f32r, name="sm",
                                              tag="sm")
                                ssum = svp.tile([128, 1], f32, name="sv",
                                                tag="sv")
                                nc.scalar.activation(sm, pas[half], AF.Exp,
                                                     bias=zerov[:, :],
                                                     scale=0.125,
                                                     accum_out=ssum)
                                isum = svp.tile([128, 1], f32, name="sv",
                                                tag="sv")
                                nc.vector.reciprocal(isum, ssum)
                                vs = vsp.tile([128, 64], f32r, name="vs",
                                              tag="vs")
                                nc.vector.tensor_scalar_mul(
                                    vs, Vv[hi][:, kt, ho:ho + 64], isum)
                                sms.append(sm)
                                vss.append(vs)
                        return sms, vss

                    def emit_av(hi, sms, vss):
                        oc = ocp.tile([128, T], bf16, name="oc", tag="oc")
                        for half in range(2):
                            ho = half * 64
                            po = pso.tile([64, T], f32, name="oh", tag="oh")
                            for kt in range(NT):
                                nc.tensor.matmul(po, vss[kt * 2 + half],
                                                 sms[kt * 2 + half],
                                                 start=(kt == 0),
                                                 stop=(kt == NT - 1))
                            nc.vector.tensor_copy(oc[ho:ho + 64, :], po)
                        OC.append(oc)

                    do_qk(0)
                    do_v(0)
                    for hi in range(NC - 1):
                        sms, vss = emit_scores(hi)
                        do_qk(hi + 1)
                        do_v(hi + 1)
                        if hi == 0:
                            wot = load_w(wo_r)  # prefetch Wo
                        emit_av(hi, sms, vss)
                    sms5, vss5 = emit_scores(NC - 1)
                    qctx.close()  # free ps_qk/ps_v banks for the projection

                    # out proj: partial accumulation (ct<5) fills the PE
                    # while the last pair's exps run on ACT
                    with tc.tile_pool(name="ps_c", bufs=3, space="PSUM") as psc:
                        NFILL = 3

                        def stt_res(py, m):
                            r = trunk.tile([128, T], f32r, name="res",
                                           tag="res", bufs=7)
                            nc.vector.scalar_tensor_tensor(
                                out=r, in0=py,
                                scalar=bo_v[:, l, m:m + 1], in1=xT[m],
                                op0=ALU.add, op1=ALU.add)
                            res1.append(r)

                        pys = []
                        for m in range(NFILL):
                            py = psc.tile([128, T], f32, name="c", tag="c")
                            for ct in range(NC - 1):
                                nc.tensor.matmul(
                                    py, wot[ct][:, m * 128:(m + 1) * 128],
                                    OC[ct], start=(ct == 0), stop=False)
                            pys.append(py)
                        emit_av(NC - 1, sms5, vss5)
                        for m in range(NFILL):
                            nc.tensor.matmul(
                                pys[m], wot[NC - 1][:, m * 128:(m + 1) * 128],
                                OC[NC - 1], start=False, stop=True)
                            stt_res(pys[m], m)
                        for m in range(NFILL, NC):
                            py = psc.tile([128, T], f32, name="c", tag="c")
                            for ct in range(NC):
                                nc.tensor.matmul(
                                    py, wot[ct][:, m * 128:(m + 1) * 128],
                                    OC[ct], start=(ct == 0),
                                    stop=(ct == NC - 1))
                            stt_res(py, m)
                xln = layernorm(res1, g1_v, be1_v, l)

                # ---- FFN ----
                w2_r = w2_d[l].rearrange("(hh p) n -> p hh n", p=128)
                res2 = []
                with tc.tile_pool(name="ps_acc", bufs=6, space="PSUM") as psd, \
                     tc.tile_pool(name="ps_h1", bufs=2, space="PSUM") as psh:
                    acc = [psd.tile([128, T], f32, name="acc", tag="acc")
                           for _ in range(NC)]

                    def emit_w1(hh):
                        w1t = w1p.tile([128, NC, 128], bf16, name="w1", tag="w1")
                        nc.sync.dma_start(out=w1t, in_=w1_d[l, hh])
                        w2t = w2p.tile([128, C], bf16, name="w2", tag="w2")
                        nc.sync.dma_start(out=w2t, in_=w2_r[:, hh, :])
                        ph = psh.tile([128, T], f32, name="h1", tag="h1")
                        for ct in range(NC):
                            nc.tensor.matmul(ph, w1t[:, ct, :], xln[ct],
                                             start=(ct == 0), stop=(ct == NC - 1))
                        h1 = h1p.tile([128, T], bf16, name="h1s", tag="h1s")
                        nc.scalar.activation(h1, ph, AF.Relu,
                                             bias=b1_v[:, l, hh:hh + 1],
                                             scale=1.0)
                        return h1, w2t

                    # software-pipelined: W1(hh+1) is emitted before W2(hh)
                    # so the PE never waits on relu(hh)
                    cur = emit_w1(0)
                    for hh in range(NFF):
                        nxt = emit_w1(hh + 1) if hh + 1 < NFF else None
                        h1, w2t = cur
                        for m in range(NC):
                            nc.tensor.matmul(acc[m], w2t[:, m * 128:(m + 1) * 128],
                                             h1, start=(hh == 0),
                                             stop=(hh == NFF - 1))
                        cur = nxt
                    for m in range(NC):
                        r = trunk.tile([128, T], f32r, name="res", tag="res",
                                       bufs=7)
                        nc.vector.scalar_tensor_tensor(
                            out=r, in0=acc[m],
                            scalar=b2_v[:, l, m:m + 1], in1=xln[m],
                            op0=ALU.add, op1=ALU.add)
                        res2.append(r)
                last = (l == n_layers - 1)
                xT = layernorm(res2, g2_v, be2_v, l,
                               out_pool=(xfp if last else None))

            xf = xT
            ctx.close()

            if debug_xt:
                xo_r = xt_o_d.rearrange("(m p) t -> p m t", p=128)
                for m in range(NC):
                    nc.sync.dma_start(out=xo_r[:, m, :], in_=xf[m])

            # ---- decoder ----
            if with_decoder:
                with tc.tile_pool(name="dwp", bufs=4) as dwp, \
                     tc.tile_pool(name="dbp", bufs=6) as dbp, \
                     tc.tile_pool(name="dop", bufs=8) as dop, \
                     tc.tile_pool(name="ps_d", bufs=6, space="PSUM") as psd2:
                    for vc in range(VCN):
                        dwt = dwp.tile([128, NC, VCW], bf16, name="dw", tag="dw")
                        nc.sync.dma_start(out=dwt, in_=decw_d[vc])
                        db1 = dbp.tile([1, VCW], f32, name="db1", tag="db1")
                        nc.sync.dma_start(
                            out=db1,
                            in_=decb_d[vc * VCW:(vc + 1) * VCW]
                            .rearrange("(a v) -> a v", a=1))
                        dbb = dbp.tile([128, VCW], f32, name="dbb", tag="dbb")
                        nc.gpsimd.partition_broadcast(dbb, db1)
                        for tn in range(NT):
                            pd = psd2.tile([128, VCW], f32, name="d", tag="d")
                            for m in range(NC):
                                nc.tensor.matmul(
                                    pd, xf[m][:, tn * 128:(tn + 1) * 128],
                                    dwt[:, m, :], start=(m == 0),
                                    stop=(m == NC - 1))
                            ot = dop.tile([128, VCW], bf16, name="do", tag="do")
                            nc.vector.tensor_add(ot, pd, dbb)
                            nc.sync.dma_start(
                                out=out_d[tn * 128:(tn + 1) * 128,
                                          vc * VCW:(vc + 1) * VCW],
                                in_=ot)

    nc.compile()
    return nc


def _get_engine(n_layers=L, with_decoder=True, debug_xt=False):
    key = (n_layers, with_decoder, debug_xt)
    if key in _ENGINE:
        return _ENGINE[key]

    import jax
    import jax.numpy as jnp
    from jax.sharding import Mesh, PartitionSpec, NamedSharding
    from jax.experimental.shard_map import shard_map
    import concourse.mybir as mybir
    from concourse import bass2jax
    from concourse.bass2jax import _bass_exec_p, install_neuronx_cc_hook

    # Persistent NEFF cache keyed on BIR bytes.
    if not getattr(bass2jax, "_neff_cache_installed", False):
        import hashlib, shutil
        _orig_compile = bass2jax.compile_bir_kernel

        def _cached_compile(ant_bir_str, compile_dir_path, neff_name="file.neff"):
            cache_dir = os.path.expanduser("~/.cache/bass_neff")
            os.makedirs(cache_dir, exist_ok=True)
            key = hashlib.sha256(
                ant_bir_str if isinstance(ant_bir_str, bytes)
                else ant_bir_str.encode()).hexdigest()
            hit = os.path.join(cache_dir, f"{key}.neff")
            out = os.path.join(compile_dir_path, neff_name)
            if os.path.exists(hit):
                shutil.copyfile(hit, out)
                return out
            res = _orig_compile(ant_bir_str, compile_dir_path, neff_name)
            try:
                shutil.copyfile(res, hit)
            except OSError:
                pass
            return res

        bass2jax.compile_bir_kernel = _cached_compile
        bass2jax._neff_cache_installed = True

    install_neuronx_cc_hook()
    nc = _build_bass(n_layers, with_decoder, debug_xt)

    partition_name = (nc.partition_id_tensor.name
                      if nc.partition_id_tensor else None)
    in_names, out_names, out_avals = [], [], []
    zero_shapes = []
    for alloc in nc.m.functions[0].allocations:
        if not isinstance(alloc, mybir.MemoryLocationSet):
            continue
        name = alloc.memorylocations[0].name
        if alloc.kind == "ExternalInput":
            if name != partition_name:
                in_names.append(name)
        elif alloc.kind == "ExternalOutput":
            out_names.append(name)
            shape = tuple(alloc.tensor_shape)
            dtype = mybir.dt.np(alloc.dtype)
            out_avals.append(jax.core.ShapedArray(shape, dtype))
            zero_shapes.append((shape, dtype))
    n_params = len(in_names)
    all_in_names = in_names + out_names
    if partition_name is not None:
        all_in_names = all_in_names + [partition_name]

    def _body(*args):
        operands = list(args)
        if partition_name is not None:
            operands.append(bass2jax.partition_id_tensor())
        outs = _bass_exec_p.bind(
            *operands,
            out_avals=tuple(out_avals),
            in_names=tuple(all_in_names),
            out_names=tuple(out_names),
            lowering_input_output_aliases=(),
            sim_require_finite=True,
            sim_require_nnan=True,
            nc=nc,
        )
        return tuple(outs)

    devices = jax.devices()[:NCORES]
    mesh = Mesh(np.asarray(devices), ("core",))
    sharded_inputs = {"x0t"}
    in_specs = tuple(
        PartitionSpec("core") if n in sharded_inputs else PartitionSpec()
        for n in in_names) + (PartitionSpec("core"),) * len(out_names)
    out_specs = (PartitionSpec("core"),) * len(out_names)
    sharded = jax.jit(shard_map(_body, mesh=mesh, in_specs=in_specs,
                                out_specs=out_specs, check_rep=False),
                      keep_unused=True)

    shard = NamedSharding(mesh, PartitionSpec("core"))
    repl = NamedSharding(mesh, PartitionSpec())
    in_shardings = {n: (shard if n in sharded_inputs else repl)
                    for n in in_names}

    def make_zeros():
        return [
            jax.device_put(
                np.zeros((NCORES * s[0], *s[1:]), dt), shard)
            for (s, dt) in zero_shapes
        ]

    eng = dict(nc=nc, in_names=in_names, out_names=out_names,
               out_avals=out_avals, sharded=sharded, mesh=mesh, shard=shard,
               in_shardings=in_shardings,
               make_zeros=make_zeros, zeros=None, dev_args=None,
               dev_args_key=None)
    _ENGINE[key] = eng
    return eng


def _host_prep(inputs):
    """Returns dict name -> per-core-stacked array [NCORES*d0, ...]."""
    import ml_dtypes
    bf = ml_dtypes.bfloat16

    ids = np.asarray(inputs["input_ids"])
    emb = np.asarray(inputs["emb"], dtype=np.float32)
    pos = np.asarray(inputs["pos"], dtype=np.float32)
    x0 = emb[ids] + pos[None, :T]                      # [B, T, C]
    x0t = np.ascontiguousarray(x0.transpose(0, 2, 1)).astype(bf)  # [B, C, T]

    Wq = np.asarray(inputs["Wq"], dtype=np.float32)
    Wk = np.asarray(inputs["Wk"], dtype=np.float32)
    Wv = np.asarray(inputs["Wv"], dtype=np.float32)
    wq = np.ascontiguousarray(Wq.transpose(0, 2, 1, 3).reshape(L, C, C))
    wk = np.ascontiguousarray(Wk.transpose(0, 2, 1, 3).reshape(L, C, C))
    wv = np.ascontiguousarray(Wv.transpose(0, 2, 1, 3).reshape(L, C, C))

    def bfc(x):
        return np.ascontiguousarray(np.asarray(x, dtype=np.float32)).astype(bf)

    def f32c(x):
        return np.ascontiguousarray(np.asarray(x, dtype=np.float32))

    shared = {
        "wq": wq.astype(bf), "wk": wk.astype(bf), "wv": wv.astype(bf),
        "wo": bfc(inputs["Wo"]),
        "w1": np.asarray(inputs["W1"], dtype=np.float32)
        .reshape(L, NC, 128, NFF, 128).transpose(0, 3, 2, 1, 4).astype(bf),
        "w2": bfc(inputs["W2"]), "bo": f32c(inputs["bo"]),
        "b1": f32c(inputs["b1"]), "b2": f32c(inputs["b2"]),
        "g1": f32c(inputs["ln1_g"]), "be1": f32c(inputs["ln1_b"]),
        "g2": f32c(inputs["ln2_g"]), "be2": f32c(inputs["ln2_b"]),
        "decw": np.asarray(inputs["dec_W"], dtype=np.float32)
        .reshape(NC, 128, VCN, VCW).transpose(2, 1, 0, 3).astype(bf),
        "decb": f32c(inputs["dec_b"]),
    }
    stacked = {"x0t": x0t.reshape(B * C, T)}
    stacked.update(shared)
    return stacked


def _run(eng, stacked, want=None):
    import jax
    key = tuple(id(stacked[name]) for name in eng["in_names"])
    if eng["dev_args_key"] != key:
        eng["dev_args"] = [
            jax.device_put(stacked[name], eng["in_shardings"][name])
            for name in eng["in_names"]]
        eng["dev_args_key"] = key
    if eng["zeros"] is None:
        eng["zeros"] = eng["make_zeros"]()
    out = eng["sharded"](*eng["dev_args"], *eng["zeros"])
    res = {}
    for i, name in enumerate(eng["out_names"]):
        if want is not None and name not in want:
            continue
        a = np.asarray(out[i])
        res[name] = a.reshape(NCORES, -1, *a.shape[1:])
    return res


_PREP_CACHE = {}


def kernel(**inputs):
    eng = _get_engine()
    pkey = tuple(id(inputs[k]) for k in sorted(inputs))
    stacked = _PREP_CACHE.get(pkey)
    if stacked is None:
        stacked = _host_prep(inputs)
        _PREP_CACHE.clear()
        _PREP_CACHE[pkey] = stacked
    res = _run(eng, stacked, want=("logits",))
    logits = res["logits"].reshape(NCORES, T, V)
    return logits.astype(np.float32)


if __name__ == "__main__":
    rng = np.random.default_rng(0)
    dummy = {
        "input_ids": rng.integers(0, V, (B, T)),
        "emb": rng.standard_normal((V, C), dtype=np.float32) * 0.02,
        "pos": rng.standard_normal((T, C), dtype=np.float32) * 0.02,
        "Wq": rng.standard_normal((L, H, C, D), dtype=np.float32) * 0.02,
        "Wk": rng.standard_normal((L, H, C, D), dtype=np.float32) * 0.02,
        "Wv": rng.standard_normal((L, H, C, D), dtype=np.float32) * 0.02,
        "Wo": rng.standard_normal((L, C, C), dtype=np.float32) * 0.02,
        "bo": np.zeros((L, C), np.float32),
        "ln1_g": np.ones((L, C), np.float32),
        "ln1_b": np.zeros((L, C), np.float32),
        "W1": rng.standard_normal((L, C, FF), dtype=np.float32) * 0.02,
        "b1": np.zeros((L, 4 * C), np.float32),
        "W2": rng.standard_normal((L, FF, C), dtype=np.float32) * 0.02,
        "b2": np.zeros((L, C), np.float32),
        "ln2_g": np.ones((L, C), np.float32),
        "ln2_b": np.zeros((L, C), np.float32),
        "dec_W": rng.standard_normal((C, V), dtype=np.float32) * 0.02,
        "dec_b": np.zeros((V,), np.float32),
    }
    out = kernel(**dummy)
    print("out", out.shape, out.dtype, float(np.abs(out).max()))
